# revision 37
# baseline (speedup 1.0000x reference)
"""LoRA linear y = x @ (B@A).T computed low-rank: y = (x @ A.T) @ B.T.

Sharding: data-parallel over tokens (B*S = 16384) across 8 NeuronCores,
2048 tokens/core; lora_A / lora_B replicated (tiny). No collectives.

All device I/O in bf16 (rel err ~3.5e-3, gate 2e-2): halves HBM traffic vs
f32 (64 MB -> 32 MB per core; ~358 GB/s/NC => ~90us floor). Host
pre-transposes x into xT chunk layout so the kernel needs NO on-device
transpose.

Tokens run through a 4-stage quarter pipeline (512 tokens each), with the
PE issue order hand-interleaved: mm1 matmuls of quarter q+1 (dep: loads,
which run ahead on the sync ring) are woven between mm2 matmul groups of
quarter q, keeping the in-order PE queue dense.

mm1 is 4x column-tiled on the PE array (tile_position=(0,32j)): the four
128-token slabs of a quarter stream concurrently through disjoint
32-column strips. Each slab's tT lands at PSUM partitions 32j..32j+16 --
exactly where mm2 wants its K=16 operands, so mm2 runs row-positioned
(tile_position=(32j,0)) against a B.T replicated at partition offsets
0/32/64/96; no partition shuffle is ever needed, and one [128,128] copy
drains a whole quarter's tT.

mm2 is 4x ROW-tiled and issued ct-interleaved: each group of 4
consecutive matmuls covers one 512-wide dout chunk for all four token
slabs at row strips 32ct (disjoint row_grps -> the PE runs them
concurrently, ~3x measured for K<=32 row tiling; the K=16 streams share
one xbus since they occupy disjoint partition ranges). This cuts mm2 PE
occupancy ~4x vs the serial same-row-group order, which previously made
y production (not HBM) the bottleneck of the whole back half.

DRAM layouts (per core, bf16):
  xtd [4q*4ld*128p, 8j*512t] : xtd[q,ld,p,j,t] = x[tok0+q*512+t, (ld*8+j)*128+p]
  atp [128, 32*16]           : atp[p, c*16+r] = A[r, c*128+p]
  btr [128, 4096]            : btr[32j+r, d] = B.T[r, d]  (x4 replicated)
  ys  [4q*4ct*128p, 4096]    : ys[q,ct,p,:] = y[tok0+q*512+ct*128+p, :]
                               (= plain row-major y for the core's tokens)

Per quarter: 4x 1MB loads -> mm1 tT[16,128]x4 (K=128, x32 accum, one full
PSUM bank, 2 rotating) -> tT to SBUF bf16 (one [128,128] copy) -> mm2 as
8 dout-chunk groups of 4 row-tiled MMs into [128,2,512] PSUM pairs (3
rotating) -> per group two [128,1024] f32->bf16 copies (DVE + ACT in
parallel) into y_sb[128,4,4096] -> 4x 1MB stores (scalar HWDGE ring) at
quarter end, double-buffered across quarters.
"""

import os
import numpy as np
import ml_dtypes

import concourse.bass as bass
import concourse.mybir as mybir
from concourse.tile import TileContext
from concourse.bass_utils import run_bass_kernel_spmd

N_CORES = 8
B, S, D_IN, D_OUT, R = 4, 4096, 4096, 4096, 16
TOK = B * S
TPC = TOK // N_CORES   # tokens per core: 2048
NQ = 4                 # quarter-pipeline stages per core
TPQ = TPC // NQ        # tokens per quarter: 512
NC_DIN = D_IN // 128   # 32 din chunks
NLD = 2                # x loads per quarter (16 chunks each, 2 MB)
JPL = NC_DIN // NLD    # din chunks per load: 16
F32 = mybir.dt.float32
BF16 = mybir.dt.bfloat16
NPBF16 = np.dtype(ml_dtypes.bfloat16)


def _split_drain_waits(nc):
    """This walrus build rejects instructions carrying >1 sem wait; hoist
    extra waits onto preceding single-wait NoOps on the same engine."""
    f = nc.m.functions[0]

    def fix_bb(bb):
        insts = getattr(bb, "instructions", None)
        if insts:
            new = []
            for inst in insts:
                si = inst.sync_info
                if si is not None and si.on_wait is not None and len(si.on_wait) > 1:
                    waits = list(si.on_wait)
                    for w in waits[:-1]:
                        d = mybir.InstNoOp(
                            name=nc.get_next_instruction_name(), ins=[], outs=[]
                        )
                        d.engine = inst.engine
                        d.sync_info = mybir.SyncInfo(on_wait=[w], on_update=[])
                        new.append(d)
                    si.on_wait = [waits[-1]]
                    inst.sync_info = si
                new.append(inst)
            bb.instructions[:] = new
        for sub in getattr(bb, "blocks", []) or []:
            fix_bb(sub)

    for blk in f.blocks:
        fix_bb(blk)


def _build():
    nc = bass.Bass("TRN2", target_bir_lowering=False, debug=False, num_devices=N_CORES)
    xtd = nc.declare_dram_parameter("xtd", [NQ * 128, NC_DIN * TPQ], BF16, isOutput=False)
    atp = nc.declare_dram_parameter("atp", [128, NC_DIN * R], BF16, isOutput=False)
    btr = nc.declare_dram_parameter("btr", [128, D_OUT], BF16, isOutput=False)
    ys = nc.declare_dram_parameter("ys", [NQ * 128, 8, 4, 512], BF16, isOutput=True)

    with TileContext(nc) as tc:
        with (
            tc.tile_pool(name="const", bufs=1) as cpool,
            tc.tile_pool(name="x", bufs=int(os.environ.get("XB", "4"))) as xpool,
            tc.tile_pool(name="t", bufs=2) as tpool,
            tc.tile_pool(name="y", bufs=int(os.environ.get("YB", "12"))) as ypool,
            tc.tile_pool(name="t_ps", bufs=2, space="PSUM") as tpsum,
            tc.tile_pool(name="y_ps0", bufs=2, space="PSUM") as ypsum0,
            tc.tile_pool(name="y_ps1", bufs=1, space="PSUM") as ypsum1,
        ):
            at_sb = cpool.tile([128, NC_DIN * R], BF16)
            nc.scalar.dma_start(out=at_sb[:], in_=atp[:])
            bt_sb = cpool.tile([128, D_OUT], BF16)
            nc.scalar.dma_start(out=bt_sb[:], in_=btr[:])
            # HAM warmup scratch: K=128 full-col matmuls on zeroed SBUF.
            # The kernel's real MMs (K=16 mm2, col-tiled mm1) never register
            # as PE activity, so without this the clock gate holds the PE at
            # 1.2 GHz for the entire kernel.
            wsc = cpool.tile([128, 512], BF16)
            nc.gpsimd.memset(wsc[:], 0.0)

            xts = {}
            xparts = {}  # q -> (n_parts, chunks_per_part)

            def issue_loads(q, parts=NLD):
                xts[q] = []
                cpp = NC_DIN // parts
                xparts[q] = (parts, cpp)
                for ld in range(parts):
                    xt = xpool.tile([128, cpp, TPQ], BF16, tag=f"xt{cpp}")
                    nc.sync.dma_start(
                        out=xt[:],
                        in_=xtd[
                            q * 128 : (q + 1) * 128,
                            ld * cpp * TPQ : (ld + 1) * cpp * TPQ,
                        ],
                    )
                    xts[q].append(xt)

            def mm1_chunk(q, tps, c):
                # one din chunk c for all 4 col-tiled 128-token slabs
                _, cpp = xparts[q]
                ld, j = c // cpp, c % cpp
                for ct in range(4):
                    nc.tensor.matmul(
                        tps[32 * ct : 32 * ct + R, 0:128],
                        at_sb[:, c * R : (c + 1) * R],
                        xts[q][ld][:, j, ct * 128 : (ct + 1) * 128],
                        start=(c == 0),
                        stop=(c == NC_DIN - 1),
                        tile_position=(0, 32 * ct),
                    )

            def keeper_mm(tgt, n=512):
                # full-array K=128 matmul on zeroed scratch into an already-
                # drained PSUM bank: numerically inert, but registers as PE
                # activity for the HAM clock gate
                nc.tensor.matmul(
                    tgt[:, 0:n],
                    wsc[:, 0:128],
                    wsc[:, 0:n],
                    start=True,
                    stop=True,
                    tile_position=(0, 0),
                )

            # prologue: q0 loads in 1MB pieces for fastest pipeline start,
            # then bt, then q1 loads (sync-ring FIFO keeps this priority);
            # HAM warmup burst runs on the PE while the loads stream
            issue_loads(0, parts=4)
            issue_loads(1)
            wps = tpsum.tile([128, 512], F32, name="tps")
            for _ in range(12):
                keeper_mm(wps)
            tps_q = {0: tpsum.tile([128, 512], F32, name="tps")}
            for c in range(NC_DIN):
                mm1_chunk(0, tps_q[0], c)

            for q in range(NQ):
                if q + 2 < NQ:
                    issue_loads(q + 2)
                # tT(q) PSUM -> SBUF bf16 (one copy; frees the bank for q+2)
                t_sb = tpool.tile([128, 128], BF16)
                if q % 2 == 0:
                    nc.vector.tensor_copy(out=t_sb[:], in_=tps_q[q][:, 0:128])
                else:
                    nc.scalar.activation(
                        out=t_sb[:], in_=tps_q[q][:, 0:128],
                        func=mybir.ActivationFunctionType.Identity,
                    )
                if q + 1 < NQ:
                    tps_q[q + 1] = tpsum.tile([128, 512], F32, name="tps")

                # mm2(q): 8 dout-chunk groups of 4 row-tiled concurrent MMs,
                # woven with mm1(q+1): 4 chunk-groups (16 col-tiled MMs) per
                # dout group
                mm1_iter = iter(list(range(NC_DIN)) if q + 1 < NQ else [])
                for h in range(8):
                    yp0 = ypsum0.tile([128, 2, 512], F32)
                    yp1 = ypsum1.tile([128, 2, 512], F32)
                    for ct in range(4):
                        yp = yp0 if ct < 2 else yp1
                        nc.tensor.matmul(
                            yp[:, ct % 2, :],
                            t_sb[32 * ct : 32 * ct + R, :],
                            bt_sb[32 * ct : 32 * ct + R, h * 512 : (h + 1) * 512],
                            start=True,
                            stop=True,
                            tile_position=(32 * ct, 0),
                        )
                    for _ in range(4):
                        nxt = next(mm1_iter, None)
                        if nxt is not None:
                            mm1_chunk(q + 1, tps_q[q + 1], nxt)
                    yh = ypool.tile([128, 4, 512], BF16)
                    # yp1 (bufs=1) gates the next group's ct2/3 MMs: drain its
                    # two banks first, one per engine, so both finish ASAP
                    nc.vector.tensor_copy(out=yh[:, 2, :], in_=yp1[:, 0, :])
                    nc.scalar.activation(
                        out=yh[:, 3, :], in_=yp1[:, 1, :],
                        func=mybir.ActivationFunctionType.Identity,
                    )
                    nc.vector.tensor_copy(out=yh[:, 0, :], in_=yp0[:, 0, :])
                    nc.scalar.activation(
                        out=yh[:, 1, :], in_=yp0[:, 1, :],
                        func=mybir.ActivationFunctionType.Identity,
                    )
                    # gpsimd SWDGE keeps store triggers off the copy engines'
                    # queues; in the last quarter the sync (load) ring is free
                    st_eng = nc.sync if (q == NQ - 1 and h % 2 == 0) else nc.gpsimd
                    st_eng.dma_start(
                        out=ys[q * 128 : (q + 1) * 128, h : h + 1, :, :],
                        in_=yh[:],
                    )

    _split_drain_waits(nc)
    return nc


_NC = None


def _get_nc():
    global _NC
    if _NC is None:
        _NC = _build()
    return _NC


def _prep_inputs(x, lora_A, lora_B):
    x_flat = np.asarray(x, dtype=np.float32).reshape(TOK, D_IN)
    xb16 = x_flat.astype(NPBF16).view(np.uint16)
    A = np.asarray(lora_A, dtype=np.float32)
    Bm = np.asarray(lora_B, dtype=np.float32)
    xtds = []
    for i in range(N_CORES):
        # xtd[q*128 + p, c*512 + t] = x[tok0 + q*512 + t, c*128 + p]
        xc = xb16[i * TPC : (i + 1) * TPC].reshape(NQ, TPQ, NC_DIN, 128)
        xtd = (
            np.ascontiguousarray(xc.transpose(0, 3, 2, 1))
            .reshape(NQ * 128, NC_DIN * TPQ)
            .view(NPBF16)
        )
        xtds.append(xtd)
    # atp[p, c*R + r] = A[r, c*128 + p]
    atp = np.ascontiguousarray(
        A.T.reshape(NC_DIN, 128, R).transpose(1, 0, 2).reshape(128, NC_DIN * R)
    ).astype(NPBF16)
    # btr[32j + r, :] = B.T[r, :], replicated at partition offsets 0/32/64/96
    btv = np.ascontiguousarray(Bm.T).astype(NPBF16)
    btrm = np.zeros((128, D_OUT), dtype=NPBF16)
    for j in range(4):
        btrm[32 * j : 32 * j + R] = btv
    return xtds, atp, btrm


def kernel(x, lora_A, lora_B, _trace=False, _trace_kwargs=None):
    nc = _get_nc()
    xtds, atp, btrm = _prep_inputs(x, lora_A, lora_B)
    in_maps = [{"xtd": xtds[i], "atp": atp, "btr": btrm} for i in range(N_CORES)]
    res = run_bass_kernel_spmd(
        nc, in_maps, list(range(N_CORES)), trace=_trace, **(_trace_kwargs or {})
    )
    out = np.empty((TOK, D_OUT), dtype=np.float32)
    for i in range(N_CORES):
        # ys[q*128+p, h, ct, t] = y[tok0 + q*512 + ct*128 + p, h*512 + t]
        u = (
            np.asarray(res.results[i]["ys"]).view(np.uint16)
            .reshape(NQ, 128, 8, 4, 512).transpose(0, 3, 1, 2, 4)
        )
        out[i * TPC : (i + 1) * TPC] = (
            np.ascontiguousarray(u).reshape(TPC, D_OUT).view(NPBF16).astype(np.float32)
        )
    out = out.reshape(B, S, D_OUT)
    if _trace:
        return out, res
    return out


# revision 39
# speedup vs baseline: 1.0652x; 1.0652x over previous
"""LoRA linear y = x @ (B@A).T computed low-rank: y = (x @ A.T) @ B.T.

Sharding: data-parallel over tokens (B*S = 16384) across 8 NeuronCores,
2048 tokens/core; lora_A / lora_B replicated (tiny). No collectives.

All device I/O in bf16 (rel err ~3.5e-3, gate 2e-2): halves HBM traffic vs
f32 (64 MB -> 32 MB per core; ~358 GB/s/NC => ~90us floor). Host
pre-transposes x into xT chunk layout so the kernel needs NO on-device
transpose.

Tokens run through a 4-stage quarter pipeline (512 tokens each), with the
PE issue order hand-interleaved: mm1 matmuls of quarter q+1 (dep: loads,
which run ahead on the sync ring) are woven between mm2 matmul groups of
quarter q, keeping the in-order PE queue dense.

mm1 is 4x column-tiled on the PE array (tile_position=(0,32j)): the four
128-token slabs of a quarter stream concurrently through disjoint
32-column strips. Each slab's tT lands at PSUM partitions 32j..32j+16 --
exactly where mm2 wants its K=16 operands, so mm2 runs row-positioned
(tile_position=(32j,0)) against a B.T replicated at partition offsets
0/32/64/96; no partition shuffle is ever needed, and one [128,128] copy
drains a whole quarter's tT.

mm2 is 4x ROW-tiled and issued ct-interleaved: each group of 4
consecutive matmuls covers one 512-wide dout chunk for all four token
slabs at row strips 32ct (disjoint row_grps -> the PE runs them
concurrently, ~3x measured for K<=32 row tiling; the K=16 streams share
one xbus since they occupy disjoint partition ranges). This cuts mm2 PE
occupancy ~4x vs the serial same-row-group order, which previously made
y production (not HBM) the bottleneck of the whole back half.

DRAM layouts (per core, bf16):
  xtd [4q*4ld*128p, 8j*512t] : xtd[q,ld,p,j,t] = x[tok0+q*512+t, (ld*8+j)*128+p]
  atp [128, 32*16]           : atp[p, c*16+r] = A[r, c*128+p]
  btr [128, 4096]            : btr[32j+r, d] = B.T[r, d]  (x4 replicated)
  ys  [4q*4ct*128p, 4096]    : ys[q,ct,p,:] = y[tok0+q*512+ct*128+p, :]
                               (= plain row-major y for the core's tokens)

Per quarter: 4x 1MB loads -> mm1 tT[16,128]x4 (K=128, x32 accum, one full
PSUM bank, 2 rotating) -> tT to SBUF bf16 (one [128,128] copy) -> mm2 as
8 dout-chunk groups of 4 row-tiled MMs into [128,2,512] PSUM pairs (3
rotating) -> per group two [128,1024] f32->bf16 copies (DVE + ACT in
parallel) into y_sb[128,4,4096] -> 4x 1MB stores (scalar HWDGE ring) at
quarter end, double-buffered across quarters.
"""

import os
import numpy as np
import ml_dtypes

import concourse.bass as bass
import concourse.mybir as mybir
from concourse.tile import TileContext
from concourse.bass_utils import run_bass_kernel_spmd

N_CORES = 8
B, S, D_IN, D_OUT, R = 4, 4096, 4096, 4096, 16
TOK = B * S
TPC = TOK // N_CORES   # tokens per core: 2048
NQ = 4                 # quarter-pipeline stages per core
TPQ = TPC // NQ        # tokens per quarter: 512
NC_DIN = D_IN // 128   # 32 din chunks
NLD = 2                # x loads per quarter (16 chunks each, 2 MB)
JPL = NC_DIN // NLD    # din chunks per load: 16
F32 = mybir.dt.float32
BF16 = mybir.dt.bfloat16
NPBF16 = np.dtype(ml_dtypes.bfloat16)


def _split_drain_waits(nc):
    """This walrus build rejects instructions carrying >1 sem wait; hoist
    extra waits onto preceding single-wait NoOps on the same engine."""
    f = nc.m.functions[0]

    def fix_bb(bb):
        insts = getattr(bb, "instructions", None)
        if insts:
            new = []
            for inst in insts:
                si = inst.sync_info
                if si is not None and si.on_wait is not None and len(si.on_wait) > 1:
                    waits = list(si.on_wait)
                    for w in waits[:-1]:
                        d = mybir.InstNoOp(
                            name=nc.get_next_instruction_name(), ins=[], outs=[]
                        )
                        d.engine = inst.engine
                        d.sync_info = mybir.SyncInfo(on_wait=[w], on_update=[])
                        new.append(d)
                    si.on_wait = [waits[-1]]
                    inst.sync_info = si
                new.append(inst)
            bb.instructions[:] = new
        for sub in getattr(bb, "blocks", []) or []:
            fix_bb(sub)

    for blk in f.blocks:
        fix_bb(blk)


def _build():
    nc = bass.Bass("TRN2", target_bir_lowering=False, debug=False, num_devices=N_CORES)
    xtd = nc.declare_dram_parameter("xtd", [NQ * 128, NC_DIN * TPQ], BF16, isOutput=False)
    atp = nc.declare_dram_parameter("atp", [128, NC_DIN * R], BF16, isOutput=False)
    btr = nc.declare_dram_parameter("btr", [128, D_OUT], BF16, isOutput=False)
    ys = nc.declare_dram_parameter("ys", [NQ * 128, 8, 4, 512], BF16, isOutput=True)

    with TileContext(nc) as tc:
        with (
            tc.tile_pool(name="const", bufs=1) as cpool,
            tc.tile_pool(name="x", bufs=int(os.environ.get("XB", "5"))) as xpool,
            tc.tile_pool(name="t", bufs=2) as tpool,
            tc.tile_pool(name="y", bufs=int(os.environ.get("YB", "12"))) as ypool,
            tc.tile_pool(name="t_ps", bufs=2, space="PSUM") as tpsum,
            tc.tile_pool(name="y_ps0", bufs=2, space="PSUM") as ypsum0,
            tc.tile_pool(name="y_ps1", bufs=1, space="PSUM") as ypsum1,
        ):
            at_sb = cpool.tile([128, NC_DIN * R], BF16)
            nc.scalar.dma_start(out=at_sb[:], in_=atp[:])
            bt_sb = cpool.tile([128, D_OUT], BF16)
            nc.scalar.dma_start(out=bt_sb[:], in_=btr[:])
            # HAM warmup scratch: K=128 full-col matmuls on zeroed SBUF.
            # The kernel's real MMs (K=16 mm2, col-tiled mm1) never register
            # as PE activity, so without this the clock gate holds the PE at
            # 1.2 GHz for the entire kernel.
            wsc = cpool.tile([128, 512], BF16)
            nc.gpsimd.memset(wsc[:], 0.0)

            xts = {}
            xparts = {}  # q -> (n_parts, chunks_per_part)

            def issue_loads(q, parts=NLD):
                xts[q] = []
                cpp = NC_DIN // parts
                xparts[q] = (parts, cpp)
                for ld in range(parts):
                    xt = xpool.tile([128, cpp, TPQ], BF16, tag=f"xt{cpp}")
                    nc.sync.dma_start(
                        out=xt[:],
                        in_=xtd[
                            q * 128 : (q + 1) * 128,
                            ld * cpp * TPQ : (ld + 1) * cpp * TPQ,
                        ],
                    )
                    xts[q].append(xt)

            def mm1_chunk(q, tps, c):
                # one din chunk c for all 4 col-tiled 128-token slabs
                _, cpp = xparts[q]
                ld, j = c // cpp, c % cpp
                for ct in range(4):
                    nc.tensor.matmul(
                        tps[32 * ct : 32 * ct + R, 0:128],
                        at_sb[:, c * R : (c + 1) * R],
                        xts[q][ld][:, j, ct * 128 : (ct + 1) * 128],
                        start=(c == 0),
                        stop=(c == NC_DIN - 1),
                        tile_position=(0, 32 * ct),
                    )

            def keeper_mm(tgt, n=512):
                # full-array K=128 matmul on zeroed scratch into an already-
                # drained PSUM bank: numerically inert, but registers as PE
                # activity for the HAM clock gate
                nc.tensor.matmul(
                    tgt[:, 0:n],
                    wsc[:, 0:128],
                    wsc[:, 0:n],
                    start=True,
                    stop=True,
                    tile_position=(0, 0),
                )

            # prologue: q0 loads in 1MB pieces for fastest pipeline start,
            # then bt, then q1 loads (sync-ring FIFO keeps this priority);
            # HAM warmup burst runs on the PE while the loads stream
            issue_loads(0)
            issue_loads(1)
            wps = tpsum.tile([128, 512], F32, name="tps")
            for _ in range(12):
                keeper_mm(wps)
            tps_q = {0: tpsum.tile([128, 512], F32, name="tps")}
            for c in range(NC_DIN):
                mm1_chunk(0, tps_q[0], c)

            for q in range(NQ):
                if q + 2 < NQ:
                    issue_loads(q + 2)
                # tT(q) PSUM -> SBUF bf16 (one copy; frees the bank for q+2)
                t_sb = tpool.tile([128, 128], BF16)
                if q % 2 == 0:
                    nc.vector.tensor_copy(out=t_sb[:], in_=tps_q[q][:, 0:128])
                else:
                    nc.scalar.activation(
                        out=t_sb[:], in_=tps_q[q][:, 0:128],
                        func=mybir.ActivationFunctionType.Identity,
                    )
                if q + 1 < NQ:
                    tps_q[q + 1] = tpsum.tile([128, 512], F32, name="tps")

                # mm2(q): 8 dout-chunk groups of 4 row-tiled concurrent MMs,
                # woven with mm1(q+1): 4 chunk-groups (16 col-tiled MMs) per
                # dout group
                mm1_iter = iter(list(range(NC_DIN)) if q + 1 < NQ else [])
                for h in range(8):
                    yp0 = ypsum0.tile([128, 2, 512], F32)
                    yp1 = ypsum1.tile([128, 2, 512], F32)
                    for ct in range(4):
                        yp = yp0 if ct < 2 else yp1
                        nc.tensor.matmul(
                            yp[:, ct % 2, :],
                            t_sb[32 * ct : 32 * ct + R, :],
                            bt_sb[32 * ct : 32 * ct + R, h * 512 : (h + 1) * 512],
                            start=True,
                            stop=True,
                            tile_position=(32 * ct, 0),
                        )
                    for _ in range(4):
                        nxt = next(mm1_iter, None)
                        if nxt is not None:
                            mm1_chunk(q + 1, tps_q[q + 1], nxt)
                    yh = ypool.tile([128, 4, 512], BF16)
                    # yp1 (bufs=1) gates the next group's ct2/3 MMs: drain its
                    # two banks first, one per engine, so both finish ASAP
                    nc.vector.tensor_copy(out=yh[:, 2, :], in_=yp1[:, 0, :])
                    nc.scalar.activation(
                        out=yh[:, 3, :], in_=yp1[:, 1, :],
                        func=mybir.ActivationFunctionType.Identity,
                    )
                    nc.vector.tensor_copy(out=yh[:, 0, :], in_=yp0[:, 0, :])
                    nc.scalar.activation(
                        out=yh[:, 1, :], in_=yp0[:, 1, :],
                        func=mybir.ActivationFunctionType.Identity,
                    )
                    # gpsimd SWDGE keeps store triggers off the copy engines'
                    # queues; in the last quarter the sync (load) ring is free
                    st_eng = nc.sync if (q == NQ - 1 and h % 2 == 0) else nc.gpsimd
                    st_eng.dma_start(
                        out=ys[q * 128 : (q + 1) * 128, h : h + 1, :, :],
                        in_=yh[:],
                    )

    _split_drain_waits(nc)
    return nc


_NC = None


def _get_nc():
    global _NC
    if _NC is None:
        _NC = _build()
    return _NC


def _prep_inputs(x, lora_A, lora_B):
    x_flat = np.asarray(x, dtype=np.float32).reshape(TOK, D_IN)
    xb16 = x_flat.astype(NPBF16).view(np.uint16)
    A = np.asarray(lora_A, dtype=np.float32)
    Bm = np.asarray(lora_B, dtype=np.float32)
    xtds = []
    for i in range(N_CORES):
        # xtd[q*128 + p, c*512 + t] = x[tok0 + q*512 + t, c*128 + p]
        xc = xb16[i * TPC : (i + 1) * TPC].reshape(NQ, TPQ, NC_DIN, 128)
        xtd = (
            np.ascontiguousarray(xc.transpose(0, 3, 2, 1))
            .reshape(NQ * 128, NC_DIN * TPQ)
            .view(NPBF16)
        )
        xtds.append(xtd)
    # atp[p, c*R + r] = A[r, c*128 + p]
    atp = np.ascontiguousarray(
        A.T.reshape(NC_DIN, 128, R).transpose(1, 0, 2).reshape(128, NC_DIN * R)
    ).astype(NPBF16)
    # btr[32j + r, :] = B.T[r, :], replicated at partition offsets 0/32/64/96
    btv = np.ascontiguousarray(Bm.T).astype(NPBF16)
    btrm = np.zeros((128, D_OUT), dtype=NPBF16)
    for j in range(4):
        btrm[32 * j : 32 * j + R] = btv
    return xtds, atp, btrm


def kernel(x, lora_A, lora_B, _trace=False, _trace_kwargs=None):
    nc = _get_nc()
    xtds, atp, btrm = _prep_inputs(x, lora_A, lora_B)
    in_maps = [{"xtd": xtds[i], "atp": atp, "btr": btrm} for i in range(N_CORES)]
    res = run_bass_kernel_spmd(
        nc, in_maps, list(range(N_CORES)), trace=_trace, **(_trace_kwargs or {})
    )
    out = np.empty((TOK, D_OUT), dtype=np.float32)
    for i in range(N_CORES):
        # ys[q*128+p, h, ct, t] = y[tok0 + q*512 + ct*128 + p, h*512 + t]
        u = (
            np.asarray(res.results[i]["ys"]).view(np.uint16)
            .reshape(NQ, 128, 8, 4, 512).transpose(0, 3, 1, 2, 4)
        )
        out[i * TPC : (i + 1) * TPC] = (
            np.ascontiguousarray(u).reshape(TPC, D_OUT).view(NPBF16).astype(np.float32)
        )
    out = out.reshape(B, S, D_OUT)
    if _trace:
        return out, res
    return out
